# revision 1
# baseline (speedup 1.0000x reference)
"""CloudCastV2 shifted-window transformer block on 8 trn2 NeuronCores.

Data-parallel over batch: 64 images -> 8 per core. Each core runs the full
block (LN1 -> shifted-window MHA -> gated residual -> LN2 -> MLP -> residual)
on its 8 images. The (-4,-4) roll + 8x8 window partition is folded into the
input/output DMA access patterns, so on chip everything lives in
"window-ordered" token space (8192 tokens x 512 ch per core).

Layouts on chip (per 256-token chunk = 4 windows):
  natural:    [128 tokens (partitions), C free]  - LN stats, skip/gate adds
  transposed: [C (partitions, 4 tiles), tokens]  - all dense matmuls (fp32r)
  attention:  qT/kT premasked bf16; per (window-pair, head) 128x128 qk^T with
              block-diag bias (-1e30 off-diag) -> exp -> ones-matmul denom ->
              reciprocal -> normalize -> AV matmul gives out^T directly.
"""

import numpy as np
import ml_dtypes

WS, SHIFT, HEADS, DIM, HRES, WRES = 8, 4, 8, 512, 32, 32
N = WS * WS            # 64 tokens / window
NH = HEADS
D = DIM // NH          # 64
B_TOTAL, NCORES = 64, 8
B_LOC = B_TOTAL // NCORES          # 8 images / core
TOK_IMG = HRES * WRES              # 1024
CHUNK = 256                        # tokens per chunk (4 windows)
NCHUNK = B_LOC * TOK_IMG // CHUNK  # 32
TT_CH = CHUNK // 128               # 128-token tiles per chunk (2)
WP_CH = TT_CH                      # window-pairs per chunk (2)
SCALE = float(D) ** -0.5
NEG = -1.0e30

F32 = None  # filled after mybir import
_prog_cache = {}


def _rel_index(ws):
    coords = np.arange(ws)
    grid = np.stack(np.meshgrid(coords, coords, indexing="ij"))
    flat = grid.reshape(2, -1)
    rel = flat[:, :, None] - flat[:, None, :]
    rel[0] += ws - 1
    rel[1] += ws - 1
    return rel[0] * (2 * ws - 1) + rel[1]


def _shift_mask(ws, shift):
    base = np.zeros((ws, ws), dtype=bool)
    base[ws - shift:, :] = True
    base[:, ws - shift:] = True
    return base.reshape(-1)


def _win_pieces(w):
    """DMA pieces for window w (0..15): list of (p0, np_, h0, q0, nq, w0).

    Window w = (wi, wj). Token (i, j) -> partition 8*i+j, source
    h=(8*wi+i+4)%32, w=(8*wj+j+4)%32. Returns pieces splitting the wrap.
    """
    wi, wj = w // 4, w % 4
    ih = [(0, 8, 8 * wi + 4)] if wi < 3 else [(0, 4, 28), (4, 4, 0)]
    jw = [(0, 8, 8 * wj + 4)] if wj < 3 else [(0, 4, 28), (4, 4, 0)]
    out = []
    for (i0, ni, h0) in ih:
        for (j0, nj, w0) in jw:
            out.append((i0, ni, h0, j0, nj, w0))
    return out


def _build_program():
    import concourse.bass as bass
    from concourse import bacc
    import concourse.mybir as mybir
    import concourse.tile as tile
    from concourse.masks import make_identity

    dt = mybir.dt
    f32, f32r, bf16 = dt.float32, dt.float32r, dt.bfloat16
    AF = mybir.ActivationFunctionType
    OP = mybir.AluOpType

    nc = bacc.Bacc("TRN2", target_bir_lowering=False, debug=True)
    x_d = nc.declare_dram_parameter("x", [B_LOC, TOK_IMG, DIM], f32, isOutput=False)
    y_d = nc.declare_dram_parameter("y", [B_LOC, TOK_IMG, DIM], f32, isOutput=True)
    wqT_d = nc.declare_dram_parameter("wqT", [DIM, DIM], f32, isOutput=False)
    wkT_d = nc.declare_dram_parameter("wkT", [DIM, DIM], f32, isOutput=False)
    wvT_d = nc.declare_dram_parameter("wvT", [DIM, DIM], f32, isOutput=False)
    wpT_d = nc.declare_dram_parameter("wpT", [DIM, DIM], f32, isOutput=False)
    w1T_d = nc.declare_dram_parameter("w1T", [DIM, 4 * DIM], f32, isOutput=False)
    w2T_d = nc.declare_dram_parameter("w2T", [4 * DIM, DIM], bf16, isOutput=False)
    bq_d = nc.declare_dram_parameter("bq", [DIM], f32, isOutput=False)
    bk_d = nc.declare_dram_parameter("bk", [DIM], f32, isOutput=False)
    bv_d = nc.declare_dram_parameter("bv", [DIM], f32, isOutput=False)
    bp_d = nc.declare_dram_parameter("bp", [DIM], f32, isOutput=False)
    b1_d = nc.declare_dram_parameter("b1", [4 * DIM], f32, isOutput=False)
    b2_d = nc.declare_dram_parameter("b2", [DIM], f32, isOutput=False)
    g1_d = nc.declare_dram_parameter("g1", [DIM], f32, isOutput=False)
    bl1_d = nc.declare_dram_parameter("bl1", [DIM], f32, isOutput=False)
    g2_d = nc.declare_dram_parameter("g2", [DIM], f32, isOutput=False)
    bl2_d = nc.declare_dram_parameter("bl2", [DIM], f32, isOutput=False)
    biasT_d = nc.declare_dram_parameter("biasT", [NH, 128, 128], f32, isOutput=False)
    qm_d = nc.declare_dram_parameter("qm", [128, CHUNK], f32, isOutput=False)
    sgw_d = nc.declare_dram_parameter("sgw", [8, 128], f32, isOutput=False)

    from contextlib import ExitStack

    with tile.TileContext(nc) as tc:
        with ExitStack() as es:
            P = lambda *a, **kw: es.enter_context(tc.tile_pool(*a, **kw))
            wts = P(name="wts", bufs=1)
            cst = P(name="cst", bufs=1)
            lnp = P(name="ln", bufs=4)
            xrp = P(name="xr", bufs=2)
            xcp = P(name="xc", bufs=1)
            xnTp = P(name="xnT", bufs=2)
            qkvp = P(name="qkv", bufs=2)
            ptp = P(name="pt", bufs=2)
            t3p = P(name="t3", bufs=2)
            rcp = P(name="rc", bufs=2)
            pnp = P(name="pn", bufs=4)
            aoTp = P(name="aoT", bufs=2)
            x2Tp = P(name="x2T", bufs=2)
            x3p = P(name="x3", bufs=2)
            xn2Tp = P(name="xn2T", bufs=2)
            h1Tp = P(name="h1T", bufs=1)
            h2Tp = P(name="h2T", bufs=2)
            yop = P(name="yo", bufs=1)
            psmm = P(name="psmm", bufs=3, space="PSUM")
            pssm = P(name="pssm", bufs=1, space="PSUM")
            psdn = P(name="psdn", bufs=1, space="PSUM")
            # ---- resident weights & constants ----
            WQ = [wts.tile([128, DIM], f32r, name=f"wq{i}") for i in range(4)]
            WK = [wts.tile([128, DIM], f32r, name=f"wk{i}") for i in range(4)]
            WV = [wts.tile([128, DIM], f32r, name=f"wv{i}") for i in range(4)]
            WP = [wts.tile([128, DIM], f32r, name=f"wp{i}") for i in range(4)]
            W1 = [wts.tile([128, 4 * DIM], f32r, name=f"w1{i}") for i in range(4)]
            for i in range(4):
                nc.gpsimd.dma_start(out=WQ[i], in_=wqT_d[128 * i:128 * (i + 1), :].bitcast(f32r))
                nc.gpsimd.dma_start(out=WK[i], in_=wkT_d[128 * i:128 * (i + 1), :].bitcast(f32r))
                nc.gpsimd.dma_start(out=WV[i], in_=wvT_d[128 * i:128 * (i + 1), :].bitcast(f32r))
                nc.gpsimd.dma_start(out=WP[i], in_=wpT_d[128 * i:128 * (i + 1), :].bitcast(f32r))
                nc.gpsimd.dma_start(out=W1[i], in_=w1T_d[128 * i:128 * (i + 1), :].bitcast(f32r))
            W2b = [wts.tile([128, DIM], bf16, name=f"w2b{i}") for i in range(16)]
            for i in range(16):
                nc.gpsimd.dma_start(out=W2b[i], in_=w2T_d[128 * i:128 * (i + 1), :])

            BIAS = [cst.tile([128, 128], f32, name=f"bias{h}") for h in range(NH)]
            for h in range(NH):
                nc.gpsimd.dma_start(out=BIAS[h], in_=biasT_d[h])
            QM = cst.tile([128, CHUNK], f32, name="qm")
            nc.gpsimd.dma_start(out=QM, in_=qm_d[:, :])
            SG = cst.tile([128, 8], f32, name="sg")
            nc.gpsimd.dma_start(out=SG, in_=sgw_d[:, :].rearrange("t p -> p t"))
            IDT = cst.tile([128, 128], f32, name="idt")
            make_identity(nc, IDT)
            ONES = cst.tile([128, 128], bf16, name="ones")
            nc.vector.memset(ONES, 1.0)
            EPS = cst.tile([128, 1], f32, name="eps")
            nc.vector.memset(EPS, 1e-5)

            def vec_sb(dram, n, name):
                t = cst.tile([128, n], f32, name=name)
                nc.gpsimd.dma_start(out=t, in_=dram[:].rearrange("(t p) -> p t", p=128))
                return t

            BQ = vec_sb(bq_d, 4, "bq")
            BK = vec_sb(bk_d, 4, "bk")
            BV = vec_sb(bv_d, 4, "bv")
            BP = vec_sb(bp_d, 4, "bp")
            B1 = vec_sb(b1_d, 16, "b1")
            B2 = vec_sb(b2_d, 4, "b2")
            G1 = vec_sb(g1_d, 4, "g1")
            BL1 = vec_sb(bl1_d, 4, "bl1")
            G2 = vec_sb(g2_d, 4, "g2")
            BL2 = vec_sb(bl2_d, 4, "bl2")

            def r32(ap):
                return ap.bitcast(f32r)

            # One-time DVE "touch" of every DMA-loaded tile: converts all
            # weight/const readiness into vector-engine program order so no
            # downstream instruction needs more than 2 sync waits.
            scr = cst.tile([128, 2048], f32, name="scr")
            touch_list = (WQ + WK + WV + WP + W1 + W2b + BIAS
                          + [QM, SG, IDT, BQ, BK, BV, BP, B1, B2,
                             G1, BL1, G2, BL2])
            for tt_ in touch_list:
                n_ = tt_.shape[-1] if len(tt_.shape) == 2 else 1
                src_ = tt_ if tt_.dtype in (f32, bf16) else tt_.bitcast(f32)
                if src_.dtype == bf16:
                    nc.vector.tensor_copy(out=scr.bitcast(bf16)[:, :n_], in_=src_)
                else:
                    nc.vector.tensor_copy(out=scr[:, :n_], in_=src_)

            def win_dma(tile_, b, w, p0, store=False):
                """window w of image b <-> tile partitions [p0:p0+64). One DMA
                per 128-token tile is issued by the caller via p0==0 path."""
                if p0 != 0:
                    return  # both windows handled in one DMA at p0 == 0
                t0 = 64 * w
                if store:
                    nc.gpsimd.dma_start(out=y_d[b, t0:t0 + 128, :], in_=tile_)
                else:
                    nc.gpsimd.dma_start(out=tile_, in_=x_d[b, t0:t0 + 128, :])

            def layer_norm(xin, tag):
                """per-token stats of xin [128, DIM] -> (x-m)*rstd (no g/b)."""
                st = lnp.tile([128, 6], f32, tag=f"st{tag}", name=f"st{tag}")
                nc.vector.bn_stats(out=st, in_=xin)
                mv = lnp.tile([128, 2], f32, tag=f"mv{tag}", name=f"mv{tag}")
                nc.vector.bn_aggr(out=mv, in_=st)
                sd = lnp.tile([128, 1], f32, tag=f"sd{tag}", name=f"sd{tag}")
                nc.scalar.activation(out=sd, in_=mv[:, 1:2], func=AF.Sqrt, bias=EPS)
                rs = lnp.tile([128, 1], f32, tag=f"rs{tag}", name=f"rs{tag}")
                nc.vector.reciprocal(out=rs, in_=sd)
                xc = xcp.tile([128, DIM], f32, tag=f"xc{tag}", name=f"xc{tag}")
                nc.vector.tensor_scalar(out=xc, in0=xin, scalar1=mv[:, 0:1],
                                        scalar2=rs, op0=OP.subtract, op1=OP.mult)
                return xc

            for ch in range(NCHUNK):
                b, qt = ch // 4, ch % 4
                wbase = 4 * qt

                # ---- load (window-ordered) + LN1 + transpose -> xnT ----
                xr = [xrp.tile([128, DIM], f32, tag=f"xr{t}", name=f"xr{t}") for t in range(TT_CH)]
                for t in range(TT_CH):
                    for k in range(2):
                        win_dma(xr[t], b, wbase + 2 * t + k, 64 * k)
                xnT = [xnTp.tile([128, CHUNK], f32r, tag=f"xnT{c}", name=f"xnT{c}") for c in range(4)]
                for t in range(TT_CH):
                    xc = layer_norm(xr[t], "1")
                    for c in range(4):
                        tp = pssm.tile([128, 128], f32, tag="tp", name="tp")
                        nc.tensor.transpose(tp, xc[:, 128 * c:128 * (c + 1)], IDT)
                        nc.vector.tensor_scalar(
                            out=xnT[c][:, 128 * t:128 * (t + 1)], in0=tp,
                            scalar1=G1[:, c:c + 1], scalar2=BL1[:, c:c + 1],
                            op0=OP.mult, op1=OP.add)

                # ---- QKV ----
                qT = [qkvp.tile([128, CHUNK], bf16, tag=f"qT{c}", name=f"qT{c}") for c in range(4)]
                kT = [qkvp.tile([128, CHUNK], bf16, tag=f"kT{c}", name=f"kT{c}") for c in range(4)]
                vN = [qkvp.tile([128, DIM], bf16, tag=f"vN{t}", name=f"vN{t}") for t in range(TT_CH)]
                for c in range(4):
                    ps = psmm.tile([128, CHUNK], f32, tag="mm", name="mm")
                    for ci in range(4):
                        nc.tensor.matmul(ps, WQ[ci][:, 128 * c:128 * (c + 1)],
                                         xnT[ci], start=(ci == 0), stop=(ci == 3))
                    tq = t3p.tile([128, CHUNK], f32, tag="tq", name="tq")
                    nc.vector.tensor_scalar(out=tq, in0=ps, scalar1=BQ[:, c:c + 1],
                                            scalar2=None, op0=OP.add)
                    nc.vector.tensor_mul(out=qT[c], in0=tq, in1=QM)
                    ps2 = psmm.tile([128, CHUNK], f32, tag="mm", name="mm")
                    for ci in range(4):
                        nc.tensor.matmul(ps2, WK[ci][:, 128 * c:128 * (c + 1)],
                                         xnT[ci], start=(ci == 0), stop=(ci == 3))
                    nc.scalar.activation(out=kT[c], in_=ps2, func=AF.Identity,
                                         bias=BK[:, c:c + 1])
                for t in range(TT_CH):
                    ps = psmm.tile([128, DIM], f32, tag="mm", name="mm")
                    for ci in range(4):
                        nc.tensor.matmul(ps, xnT[ci][:, 128 * t:128 * (t + 1)],
                                         WV[ci], start=(ci == 0), stop=(ci == 3))
                    nc.scalar.activation(out=vN[t], in_=ps, func=AF.Copy)

                # ---- attention ----
                aoT = [aoTp.tile([128, CHUNK], f32r, tag=f"aoT{c}", name=f"aoT{c}") for c in range(4)]
                for wp in range(WP_CH):
                    PT = ptp.tile([128, NH * 128], bf16, tag="pt", name="pt")
                    for h in range(NH):
                        cth, ro = h // 2, 64 * (h % 2)
                        sl = slice(128 * wp, 128 * (wp + 1))
                        qk = pssm.tile([128, 128], f32, tag="qk", name="qk")
                        nc.tensor.matmul(qk, kT[cth][ro:ro + 64, sl],
                                         qT[cth][ro:ro + 64, sl], start=True, stop=True)
                        t3 = t3p.tile([128, 128], f32, tag="t3", name="t3")
                        nc.vector.scalar_tensor_tensor(
                            out=t3, in0=qk, scalar=SCALE, in1=BIAS[h],
                            op0=OP.mult, op1=OP.add)
                        nc.scalar.activation(out=PT[:, 128 * h:128 * (h + 1)],
                                             in_=t3, func=AF.Exp)
                    rc = []
                    for g in range(2):
                        dn = psdn.tile([128, 512], f32, tag="dn", name="dn")
                        nc.tensor.matmul(dn, ONES, PT[:, 512 * g:512 * (g + 1)],
                                         start=True, stop=True)
                        r = rcp.tile([128, 512], bf16, tag=f"rc{g}", name=f"rc{g}")
                        with nc.allow_low_precision(reason="attn weights bf16"):
                            nc.vector.reciprocal(out=r, in_=dn)
                        rc.append(r)
                    for h in range(NH):
                        cth, ro = h // 2, 64 * (h % 2)
                        rcb = rc[h // 4][:, 128 * (h % 4):128 * (h % 4 + 1)]
                        pn = pnp.tile([128, 128], bf16, tag="pn", name="pn")
                        nc.gpsimd.tensor_mul(out=pn, in0=PT[:, 128 * h:128 * (h + 1)],
                                             in1=rcb)
                        av = pssm.tile([128, 128], f32, tag="av", name="av")
                        nc.tensor.matmul(av[ro:ro + 64, :],
                                         vN[wp][:, 64 * h:64 * (h + 1)], pn,
                                         start=True, stop=True,
                                         tile_position=(0, ro))
                        nc.vector.tensor_scalar(
                            out=aoT[cth][ro:ro + 64, 128 * wp:128 * (wp + 1)],
                            in0=av[ro:ro + 64, :], scalar1=BV[ro:ro + 64, cth:cth + 1],
                            scalar2=None, op0=OP.add)

                # ---- proj + residual (in T) ----
                x2T = [x2Tp.tile([128, CHUNK], f32, tag=f"x2T{c}", name=f"x2T{c}") for c in range(4)]
                for c in range(4):
                    ps = psmm.tile([128, CHUNK], f32, tag="mm", name="mm")
                    for ci in range(4):
                        nc.tensor.matmul(ps, WP[ci][:, 128 * c:128 * (c + 1)],
                                         aoT[ci], start=(ci == 0), stop=(ci == 3))
                    nc.vector.scalar_tensor_tensor(
                        out=x2T[c], in0=ps, scalar=BP[:, c:c + 1], in1=xnT[c],
                        op0=OP.add, op1=OP.add)

                # ---- back to natural: x3 = x2 + sig(gate)*x ----
                x3 = [x3p.tile([128, DIM], f32, tag=f"x3{t}", name=f"x3{t}") for t in range(TT_CH)]
                for c in range(4):
                    for t in range(TT_CH):
                        tp = pssm.tile([128, 128], f32, tag="tp", name="tp")
                        nc.tensor.transpose(tp, x2T[c][:, 128 * t:128 * (t + 1)], IDT)
                        col = 2 * qt + t
                        nc.vector.scalar_tensor_tensor(
                            out=x3[t][:, 128 * c:128 * (c + 1)],
                            in0=xr[t][:, 128 * c:128 * (c + 1)],
                            scalar=SG[:, col:col + 1], in1=tp,
                            op0=OP.mult, op1=OP.add)

                # ---- LN2 + transpose ----
                xn2T = [xn2Tp.tile([128, CHUNK], f32r, tag=f"xn2T{c}", name=f"xn2T{c}") for c in range(4)]
                for t in range(TT_CH):
                    xc2 = layer_norm(x3[t], "2")
                    for c in range(4):
                        tp = pssm.tile([128, 128], f32, tag="tp", name="tp")
                        nc.tensor.transpose(tp, xc2[:, 128 * c:128 * (c + 1)], IDT)
                        nc.vector.tensor_scalar(
                            out=xn2T[c][:, 128 * t:128 * (t + 1)], in0=tp,
                            scalar1=G2[:, c:c + 1], scalar2=BL2[:, c:c + 1],
                            op0=OP.mult, op1=OP.add)

                # ---- MLP ----
                h1 = [h1Tp.tile([128, CHUNK], bf16, tag=f"h1_{o}", name=f"h1_{o}") for o in range(16)]
                for o in range(16):
                    ps = psmm.tile([128, CHUNK], f32, tag="mm", name="mm")
                    for ci in range(4):
                        nc.tensor.matmul(ps, W1[ci][:, 128 * o:128 * (o + 1)],
                                         xn2T[ci], start=(ci == 0), stop=(ci == 3))
                    nc.scalar.activation(out=h1[o], in_=ps, func=AF.Gelu,
                                         bias=B1[:, o:o + 1])
                h2T = [h2Tp.tile([128, CHUNK], f32, tag=f"h2T{c}", name=f"h2T{c}") for c in range(4)]
                for c in range(4):
                    ps = psmm.tile([128, CHUNK], f32, tag="mm", name="mm")
                    for hi in range(16):
                        nc.tensor.matmul(ps, W2b[hi][:, 128 * c:128 * (c + 1)],
                                         h1[hi], start=(hi == 0), stop=(hi == 15))
                    nc.scalar.activation(out=h2T[c], in_=ps, func=AF.Identity,
                                         bias=B2[:, c:c + 1])

                # ---- final add + store ----
                for t in range(TT_CH):
                    yo = yop.tile([128, DIM], f32, tag=f"yo{t}", name=f"yo{t}")
                    for c in range(4):
                        tp = pssm.tile([128, 128], f32, tag="tp", name="tp")
                        nc.tensor.transpose(tp, h2T[c][:, 128 * t:128 * (t + 1)], IDT)
                        nc.vector.tensor_add(out=yo[:, 128 * c:128 * (c + 1)],
                                             in0=tp, in1=x3[t][:, 128 * c:128 * (c + 1)])
                    for k in range(2):
                        win_dma(yo, b, wbase + 2 * t + k, 64 * k, store=True)

    nc.compile()
    return nc


def _host_consts(rel_table):
    idx = _rel_index(WS).reshape(-1)
    bias = rel_table.reshape(-1, NH)[idx].reshape(N, NH, N)  # [n, h, m]
    qmask = _shift_mask(WS, SHIFT)                           # [64] True=masked
    keep = (~qmask).astype(np.float32)
    biasT = np.full((NH, 128, 128), NEG, np.float32)
    for h in range(NH):
        bT = bias[:, h, :].T * keep[None, :]                 # [m, n] masked cols->0
        biasT[h, :64, :64] = bT
        biasT[h, 64:, 64:] = bT
    qm = np.tile(keep, CHUNK // N)[None, :].repeat(128, 0).astype(np.float32)
    return biasT, qm


def _win_order_sigmoid_gate(gate):
    g = 1.0 / (1.0 + np.exp(-gate.reshape(HRES, WRES).astype(np.float64)))
    g = g.astype(np.float32)
    sg = np.zeros((16, 64), np.float32)
    for w in range(16):
        wi, wj = w // 4, w % 4
        for i in range(8):
            for j in range(8):
                sg[w, 8 * i + j] = g[(8 * wi + i + 4) % 32, (8 * wj + j + 4) % 32]
    return sg.reshape(8, 128)


_PERM = None


def _perm_idx():
    global _PERM
    if _PERM is None:
        p = np.zeros(1024, np.int64)
        for w in range(16):
            for (i0, ni, h0, j0, nj, w0) in _win_pieces(w):
                for a in range(ni):
                    for bb in range(nj):
                        p[64 * w + 8 * (i0 + a) + (j0 + bb)] = (h0 + a) * WRES + (w0 + bb)
        _PERM = p
    return _PERM


def kernel(**inputs):
    from concourse.bass_utils import run_bass_kernel_spmd

    x = np.asarray(inputs["x"], np.float32)           # (64,1,32,32,512)
    biasT, qm = _host_consts(np.asarray(inputs["rel_table"], np.float32))
    sgw = _win_order_sigmoid_gate(np.asarray(inputs["gate"], np.float32))
    common = {
        "wqT": np.ascontiguousarray(np.asarray(inputs["wq"], np.float32).T),
        "wkT": np.ascontiguousarray(np.asarray(inputs["wk"], np.float32).T),
        "wvT": np.ascontiguousarray(np.asarray(inputs["wv"], np.float32).T),
        "wpT": np.ascontiguousarray(np.asarray(inputs["wp"], np.float32).T),
        "w1T": np.ascontiguousarray(np.asarray(inputs["mlp_w1"], np.float32).T),
        "w2T": np.ascontiguousarray(np.asarray(inputs["mlp_w2"], np.float32).T).astype(ml_dtypes.bfloat16),
        "bq": np.asarray(inputs["bq"], np.float32),
        "bk": np.asarray(inputs["bk"], np.float32),
        "bv": np.asarray(inputs["bv"], np.float32),
        "bp": np.asarray(inputs["bp"], np.float32),
        "b1": np.asarray(inputs["mlp_b1"], np.float32),
        "b2": np.asarray(inputs["mlp_b2"], np.float32),
        "g1": np.asarray(inputs["ln1_g"], np.float32),
        "bl1": np.asarray(inputs["ln1_b"], np.float32),
        "g2": np.asarray(inputs["ln2_g"], np.float32),
        "bl2": np.asarray(inputs["ln2_b"], np.float32),
        "biasT": biasT, "qm": qm, "sgw": sgw,
    }
    if "prog" not in _prog_cache:
        _prog_cache["prog"] = _build_program()
    nc = _prog_cache["prog"]

    perm = _perm_idx()
    xw = x.reshape(B_TOTAL, TOK_IMG, DIM)[:, perm, :]   # window-ordered
    in_maps = []
    for c in range(NCORES):
        m = dict(common)
        m["x"] = np.ascontiguousarray(xw[c * B_LOC:(c + 1) * B_LOC])
        in_maps.append(m)
    res = run_bass_kernel_spmd(nc, in_maps, core_ids=list(range(NCORES)))
    yw = np.concatenate([res.results[c]["y"] for c in range(NCORES)], axis=0)
    out = np.empty((B_TOTAL, TOK_IMG, DIM), np.float32)
    out[:, perm, :] = yw
    return out.reshape(B_TOTAL, 1, HRES, WRES, DIM).astype(np.float32)



# revision 41
# speedup vs baseline: 1.2962x; 1.2962x over previous
"""CloudCastV2 shifted-window transformer block on 8 trn2 NeuronCores.

Data-parallel over batch: 64 images -> 8 per core. Each core runs the full
block (LN1 -> shifted-window MHA -> gated residual -> LN2 -> MLP -> residual)
on its 8 images. The (-4,-4) roll + 8x8 window partition is folded into the
input/output DMA access patterns, so on chip everything lives in
"window-ordered" token space (8192 tokens x 512 ch per core).

Key structure (v2):
  - LN affines folded into the QKV/MLP weights on the host; the attention-path
    per-channel constant (bp + Wp@bv_eff + ln1_b) is injected into the proj
    PSUM via a rank-1 ones matmul, so PSUM evictions are single fused ops.
  - All weights + on-chip activations bf16 except the residual stream (f32).
  - Attention batched per 4-head group: qk^T lands in one [128,512] PSUM bank,
    bias added in-place (DVE), one exp (Act) per group.
  - The 4 per-channel-block transposes of each 128-token tile share one
    [128,512] PSUM bank and leave via one strided eviction op.
  - rstd = exp(-0.5*ln(var+eps)) keeps LN + attention exp in one activation
    table set; only Gelu swaps tables (2 swaps/chunk).
  - Input/output DMAs issued from the SP engine (HWDGE), weights from gpsimd.
"""

import numpy as np
import ml_dtypes

WS, SHIFT, HEADS, DIM, HRES, WRES = 8, 4, 8, 512, 32, 32
N = WS * WS            # 64 tokens / window
NH = HEADS
D = DIM // NH          # 64
B_TOTAL, NCORES = 64, 8
B_LOC = B_TOTAL // NCORES          # 8 images / core
TOK_IMG = HRES * WRES              # 1024
CHUNK = 256                        # tokens per chunk (4 windows)
NCHUNK = B_LOC * TOK_IMG // CHUNK  # 32
TT_CH = CHUNK // 128               # 128-token tiles per chunk (2)
WP_CH = TT_CH                      # window-pairs per chunk (2)
SCALE = float(D) ** -0.5
NEG = -1.0e30

_prog_cache = {}


def _rel_index(ws):
    coords = np.arange(ws)
    grid = np.stack(np.meshgrid(coords, coords, indexing="ij"))
    flat = grid.reshape(2, -1)
    rel = flat[:, :, None] - flat[:, None, :]
    rel[0] += ws - 1
    rel[1] += ws - 1
    return rel[0] * (2 * ws - 1) + rel[1]


def _shift_mask(ws, shift):
    base = np.zeros((ws, ws), dtype=bool)
    base[ws - shift:, :] = True
    base[:, ws - shift:] = True
    return base.reshape(-1)


def _build_program():
    import concourse.bass as bass
    from concourse import bacc
    import concourse.mybir as mybir
    import concourse.tile as tile
    from concourse.masks import make_identity

    dt = mybir.dt
    f32, f32r, bf16 = dt.float32, dt.float32r, dt.bfloat16
    AF = mybir.ActivationFunctionType
    OP = mybir.AluOpType

    nc = bacc.Bacc("TRN2", target_bir_lowering=False, debug=True)
    x_d = nc.declare_dram_parameter("x", [B_LOC, TOK_IMG, DIM], f32, isOutput=False)
    y_d = nc.declare_dram_parameter("y", [B_LOC, TOK_IMG, DIM], f32, isOutput=True)
    wqT_d = nc.declare_dram_parameter("wqT", [DIM, DIM], bf16, isOutput=False)
    wkT_d = nc.declare_dram_parameter("wkT", [DIM, DIM], bf16, isOutput=False)
    wvT_d = nc.declare_dram_parameter("wvT", [DIM, DIM], bf16, isOutput=False)
    wpT_d = nc.declare_dram_parameter("wpT", [DIM, DIM], bf16, isOutput=False)
    w1T_d = nc.declare_dram_parameter("w1T", [DIM, 4 * DIM], bf16, isOutput=False)
    w2T_d = nc.declare_dram_parameter("w2T", [4 * DIM, DIM], bf16, isOutput=False)
    bq_d = nc.declare_dram_parameter("bq", [DIM], f32, isOutput=False)
    bke_d = nc.declare_dram_parameter("bke", [DIM], f32, isOutput=False)
    bko_d = nc.declare_dram_parameter("bko", [DIM], f32, isOutput=False)
    msk_d = nc.declare_dram_parameter("msk", [128, 2], f32, isOutput=False)
    b1_d = nc.declare_dram_parameter("b1", [4 * DIM], f32, isOutput=False)
    b2_d = nc.declare_dram_parameter("b2", [DIM], f32, isOutput=False)
    g1_d = nc.declare_dram_parameter("g1", [DIM], f32, isOutput=False)
    bc_d = nc.declare_dram_parameter("bc", [4, 128, 128], bf16, isOutput=False)  # diag(bconst)
    biasG_d = nc.declare_dram_parameter("biasG", [2, 128, 512], bf16, isOutput=False)
    qm_d = nc.declare_dram_parameter("qm", [128, CHUNK], bf16, isOutput=False)
    idt_d = nc.declare_dram_parameter("idt", [128, 128], f32, isOutput=False)
    sgw_d = nc.declare_dram_parameter("sgw", [8, 128], f32, isOutput=False)

    from contextlib import ExitStack

    with tile.TileContext(nc) as tc:
        with ExitStack() as es:
            P = lambda *a, **kw: es.enter_context(tc.tile_pool(*a, **kw))
            wts = P(name="wts", bufs=1)
            cst = P(name="cst", bufs=1)
            lnp = P(name="ln", bufs=4)
            xrp = P(name="xr", bufs=3)
            xcp = P(name="xc", bufs=2)
            xnTp = P(name="xnT", bufs=2)
            qkvp = P(name="qkv", bufs=2)
            ptp = P(name="pt", bufs=2)
            t3p = P(name="t3", bufs=2)
            rcp = P(name="rc", bufs=2)
            pnp = P(name="pn", bufs=2)
            aoTp = P(name="aoT", bufs=2)
            x2Tp = P(name="x2T", bufs=2)
            x3p = P(name="x3", bufs=2)
            xn2Tp = P(name="xn2T", bufs=2)
            h1Tp = P(name="h1T", bufs=2)
            h2Tp = P(name="h2T", bufs=2)
            yop = P(name="yo", bufs=2)
            # PSUM: 8 banks total. mm 3 (big matmuls + dn), qk 2,
            # tp1 1 (LN1 transposes), tr 2 (av/TX/TP2/TY).
            psmm = P(name="psmm", bufs=3, space="PSUM")
            psqk = P(name="psqk", bufs=2, space="PSUM")
            pstp1 = P(name="pstp1", bufs=1, space="PSUM")
            pstr = P(name="pstr", bufs=2, space="PSUM")

            # ---- resident weights & constants ----
            WQ = [wts.tile([128, DIM], bf16, name=f"wq{i}") for i in range(4)]
            WK = [wts.tile([128, DIM], bf16, name=f"wk{i}") for i in range(4)]
            WV = [wts.tile([128, DIM], bf16, name=f"wv{i}") for i in range(4)]
            WP = [wts.tile([128, DIM], bf16, name=f"wp{i}") for i in range(4)]
            W1 = [wts.tile([128, 4 * DIM], bf16, name=f"w1{i}") for i in range(4)]
            for i in range(4):
                nc.gpsimd.dma_start(out=WQ[i], in_=wqT_d[128 * i:128 * (i + 1), :])
                nc.gpsimd.dma_start(out=WK[i], in_=wkT_d[128 * i:128 * (i + 1), :])
                nc.gpsimd.dma_start(out=WV[i], in_=wvT_d[128 * i:128 * (i + 1), :])
                nc.gpsimd.dma_start(out=WP[i], in_=wpT_d[128 * i:128 * (i + 1), :])
                nc.gpsimd.dma_start(out=W1[i], in_=w1T_d[128 * i:128 * (i + 1), :])
            W2b = [wts.tile([128, DIM], bf16, name=f"w2b{i}") for i in range(16)]
            for i in range(16):
                nc.gpsimd.dma_start(out=W2b[i], in_=w2T_d[128 * i:128 * (i + 1), :])

            BIASG = [cst.tile([128, 512], bf16, name=f"biasg{g}") for g in range(2)]
            for g in range(2):
                nc.gpsimd.dma_start(out=BIASG[g], in_=biasG_d[g])
            QM = cst.tile([128, CHUNK], bf16, name="qm")
            nc.gpsimd.dma_start(out=QM, in_=qm_d[:, :])
            SG = cst.tile([128, 8], f32, name="sg")
            nc.gpsimd.dma_start(out=SG, in_=sgw_d[:, :].rearrange("t p -> p t"))
            BC = [cst.tile([128, 128], bf16, name=f"bc{c}") for c in range(4)]
            for c in range(4):
                nc.gpsimd.dma_start(out=BC[c], in_=bc_d[c])
            IDTB = cst.tile([128, 128], bf16, name="idtb")
            make_identity(nc, IDTB)
            IDTR = cst.tile([128, 128], f32r, name="idtr")
            nc.gpsimd.dma_start(out=IDTR, in_=idt_d[:, :].bitcast(f32r))
            ONES = cst.tile([128, 128], bf16, name="ones")
            nc.vector.memset(ONES, 1.0)
            ONES2 = cst.tile([128, CHUNK], bf16, name="ones2")
            nc.vector.memset(ONES2, 1.0)
            EPS = cst.tile([128, 1], f32, name="eps")
            nc.vector.memset(EPS, 1e-5)

            def vec_sb(dram, n, name):
                t = cst.tile([128, n], f32, name=name)
                nc.gpsimd.dma_start(out=t, in_=dram[:].rearrange("(t p) -> p t", p=128))
                return t

            BQ = vec_sb(bq_d, 4, "bq")
            BKE = vec_sb(bke_d, 4, "bke")
            BKO = vec_sb(bko_d, 4, "bko")
            MSK = cst.tile([128, 2], f32, name="msk")
            nc.gpsimd.dma_start(out=MSK, in_=msk_d[:, :])
            B1 = vec_sb(b1_d, 16, "b1")
            B2 = vec_sb(b2_d, 4, "b2")
            G1 = vec_sb(g1_d, 4, "g1")

            # One-time DVE "touch" of every DMA-loaded tile: converts all
            # weight/const readiness into vector-engine program order so no
            # downstream instruction needs more than 2 sync waits.
            scr = cst.tile([128, 2048], f32, name="scr")
            touch_list = (WQ + WK + WV + WP + W1 + W2b + BIASG + BC
                          + [QM, SG, BQ, BKE, BKO, MSK, B1, B2, G1])
            for tt_ in touch_list:
                n_ = tt_.shape[-1] if len(tt_.shape) == 2 else 1
                if tt_.dtype == bf16:
                    nc.vector.tensor_copy(out=scr.bitcast(bf16)[:tt_.shape[0], :n_], in_=tt_)
                else:
                    nc.vector.tensor_copy(out=scr[:tt_.shape[0], :n_], in_=tt_)

            def layer_norm_rstd(xin, tag):
                """per-token mean + rstd of xin [128, DIM] via Ln/Exp."""
                st = lnp.tile([128, 6], f32, tag=f"st{tag}", name=f"st{tag}")
                nc.vector.bn_stats(out=st, in_=xin)
                mv = lnp.tile([128, 2], f32, tag=f"mv{tag}", name=f"mv{tag}")
                nc.vector.bn_aggr(out=mv, in_=st)
                lv = lnp.tile([128, 1], f32, tag=f"lv{tag}", name=f"lv{tag}")
                nc.scalar.activation(out=lv, in_=mv[:, 1:2], func=AF.Ln, bias=EPS)
                rs = lnp.tile([128, 1], f32, tag=f"rs{tag}", name=f"rs{tag}")
                nc.scalar.activation(out=rs, in_=lv, func=AF.Exp, scale=-0.5)
                return mv, rs

            def r3(t, c=4):
                return t.rearrange("p (c q) -> p c q", c=c)

            import os
            n_chunks = int(os.environ.get("K_NCHUNK", str(NCHUNK)))
            k_stage = int(os.environ.get("K_STAGE", "9"))
            for ch in range(n_chunks):
                b, qt = ch // 4, ch % 4

                # ---- load (window-ordered) + LN1 + transpose -> xnT ----
                xr = [xrp.tile([128, DIM], f32, tag=f"xr{t}", name=f"xr{t}") for t in range(TT_CH)]
                for t in range(TT_CH):
                    nc.gpsimd.dma_start(out=xr[t], in_=x_d[b, 256 * qt + 128 * t: 256 * qt + 128 * (t + 1), :])
                # xnT: [128, 1024] bf16, c-major blocks of 256 (= 2 t-tiles of 128)
                xnT = xnTp.tile([128, 4 * CHUNK], bf16, tag="xnT", name="xnT")
                for t in range(TT_CH):
                    mv, rs = layer_norm_rstd(xr[t], "1")
                    xc = xcp.tile([128, DIM], f32r, tag=f"xc{t}", name=f"xc{t}")
                    nc.vector.tensor_scalar(out=xc, in0=xr[t], scalar1=mv[:, 0:1],
                                            scalar2=rs, op0=OP.subtract, op1=OP.mult)
                    TP = pstp1.tile([128, 512], f32r, tag="tp1", name="tp1")
                    for c in range(4):
                        nc.tensor.transpose(TP[:, 128 * c:128 * (c + 1)],
                                            xc[:, 128 * c:128 * (c + 1)], IDTR)
                    # one strided eviction: TP c-blocks -> xnT[:, 256c+128t : +128]
                    nc.scalar.activation(
                        out=r3(xnT, 4)[:, :, 128 * t:128 * (t + 1)],
                        in_=r3(TP.bitcast(f32), 4), func=AF.Copy)

                def xnTc(c):
                    return xnT[:, CHUNK * c:CHUNK * (c + 1)]

                if k_stage < 2:
                    continue

                # ---- QKV ----
                qT = [qkvp.tile([128, CHUNK], bf16, tag=f"qT{c}", name=f"qT{c}") for c in range(4)]
                kTE = [qkvp.tile([128, CHUNK], bf16, tag=f"kTE{c}", name=f"kTE{c}") for c in range(4)]
                kTO = [qkvp.tile([128, CHUNK], bf16, tag=f"kTO{c}", name=f"kTO{c}") for c in range(4)]
                vN = [qkvp.tile([128, DIM], bf16, tag=f"vN{t}", name=f"vN{t}") for t in range(TT_CH)]
                for c in range(4):
                    ps = psmm.tile([128, CHUNK], f32, tag="mm", name="mm")
                    for ci in range(4):
                        nc.tensor.matmul(ps, WQ[ci][:, 128 * c:128 * (c + 1)],
                                         xnTc(ci), start=(ci == 0), stop=(ci == 3))
                    # qT = (ps + bq) * (SCALE*keep)
                    nc.vector.scalar_tensor_tensor(
                        out=qT[c], in0=ps, scalar=BQ[:, c:c + 1], in1=QM,
                        op0=OP.add, op1=OP.mult)
                    ps2 = psmm.tile([128, CHUNK], f32, tag="mm", name="mm")
                    for ci in range(4):
                        nc.tensor.matmul(ps2, WK[ci][:, 128 * c:128 * (c + 1)],
                                         xnTc(ci), start=(ci == 0), stop=(ci == 3))
                    # parity-masked kT: even-head rows / odd-head rows only,
                    # so qk matmuls can contract over the full 128 partitions
                    nc.scalar.activation(out=kTE[c], in_=ps2, func=AF.Identity,
                                         scale=MSK[:, 0:1], bias=BKE[:, c:c + 1])
                    nc.vector.tensor_scalar(out=kTO[c], in0=ps2,
                                            scalar1=BKO[:, c:c + 1],
                                            scalar2=MSK[:, 1:2],
                                            op0=OP.add, op1=OP.mult)
                for t in range(TT_CH):
                    ps = psmm.tile([128, DIM], f32, tag="mm", name="mm")
                    for ci in range(4):
                        nc.tensor.matmul(ps, xnT[:, CHUNK * ci + 128 * t: CHUNK * ci + 128 * (t + 1)],
                                         WV[ci], start=(ci == 0), stop=(ci == 3))
                    nc.scalar.activation(out=vN[t], in_=ps, func=AF.Copy)

                if k_stage < 3:
                    continue
                k_attn = int(os.environ.get("K_ATTN", "4"))
                # ---- attention ----
                # aoT: [128, 1024] bf16, c-major blocks of 256 (= 2 wp-tiles of 128)
                aoT = aoTp.tile([128, 4 * CHUNK], bf16, tag="aoT", name="aoT")
                for wp in range(WP_CH):
                    PT = ptp.tile([128, NH * 128], bf16, tag="pt", name="pt")
                    sl = slice(128 * wp, 128 * (wp + 1))
                    for g in range(2):
                        qk = psqk.tile([128, 512], f32, tag="qk", name="qk")
                        for hh in range(4):
                            h = 4 * g + hh
                            cth = h // 2
                            kTz = kTE[cth] if h % 2 == 0 else kTO[cth]
                            nc.tensor.matmul(qk[:, 128 * hh:128 * (hh + 1)],
                                             kTz[:, sl], qT[cth][:, sl],
                                             start=True, stop=True)
                        # bias add -> SBUF, then one exp for the 4-head group
                        t3 = t3p.tile([128, 512], f32, tag="t3", name="t3")
                        nc.vector.tensor_tensor(out=t3, in0=qk, in1=BIASG[g],
                                                op=OP.add)
                        nc.scalar.activation(out=PT[:, 512 * g:512 * (g + 1)],
                                             in_=t3, func=AF.Exp)
                    if k_attn < 2:
                        continue
                    pn = pnp.tile([128, NH * 128], bf16, tag="pn", name="pn")
                    for g in range(2):
                        dn = psmm.tile([128, 512], f32, tag="mm", name="mm")
                        nc.tensor.matmul(dn, ONES, PT[:, 512 * g:512 * (g + 1)],
                                         start=True, stop=True)
                        r = rcp.tile([128, 512], bf16, tag=f"rc{g}", name=f"rc{g}")
                        with nc.allow_low_precision(reason="attn weights bf16"):
                            nc.vector.reciprocal(out=r, in_=dn)
                        if k_attn < 3:
                            continue
                        nc.gpsimd.tensor_mul(out=pn[:, 512 * g:512 * (g + 1)],
                                             in0=PT[:, 512 * g:512 * (g + 1)], in1=r)
                    if k_attn < 4:
                        continue
                    av = pstr.tile([128, 512], f32, tag="tr", name="av")
                    for h in range(NH):
                        cth, ro = h // 2, 64 * (h % 2)
                        nc.tensor.matmul(av[ro:ro + 64, 128 * cth:128 * (cth + 1)],
                                         vN[wp][:, 64 * h:64 * (h + 1)],
                                         pn[:, 128 * h:128 * (h + 1)],
                                         start=True, stop=True,
                                         tile_position=(0, ro))
                    nc.scalar.activation(
                        out=r3(aoT, 4)[:, :, 128 * wp:128 * (wp + 1)],
                        in_=r3(av, 4), func=AF.Copy)

                if k_stage < 4:
                    continue
                # ---- proj + rank-1 bias + residual (in T) ----
                x2T = [x2Tp.tile([128, CHUNK], f32r, tag=f"x2T{c}", name=f"x2T{c}") for c in range(4)]
                for c in range(4):
                    ps = psmm.tile([128, CHUNK], f32, tag="mm", name="mm")
                    for ci in range(4):
                        nc.tensor.matmul(ps, WP[ci][:, 128 * c:128 * (c + 1)],
                                         aoT[:, CHUNK * ci:CHUNK * (ci + 1)],
                                         start=(ci == 0), stop=False)
                    nc.tensor.matmul(ps, BC[c], ONES2,
                                     start=False, stop=True)
                    # x2T = g1 (.) xnT + (proj + bconst)
                    nc.vector.scalar_tensor_tensor(
                        out=x2T[c], in0=xnTc(c), scalar=G1[:, c:c + 1], in1=ps,
                        op0=OP.mult, op1=OP.add)

                if k_stage < 5:
                    continue
                # ---- back to natural: x3 = x2 + sig(gate)*x ----
                x3 = [x3p.tile([128, DIM], f32, tag=f"x3{t}", name=f"x3{t}") for t in range(TT_CH)]
                for t in range(TT_CH):
                    TX = pstr.tile([128, 512], f32r, tag="tr", name="tx")
                    for c in range(4):
                        nc.tensor.transpose(TX[:, 128 * c:128 * (c + 1)],
                                            x2T[c][:, 128 * t:128 * (t + 1)],
                                            IDTR)
                    col = 2 * qt + t
                    nc.vector.scalar_tensor_tensor(
                        out=x3[t], in0=xr[t], scalar=SG[:, col:col + 1],
                        in1=TX.bitcast(f32), op0=OP.mult, op1=OP.add)

                if k_stage < 6:
                    continue
                # ---- LN2 + transpose (g2/b2 folded into W1/b1) ----
                xn2T = xn2Tp.tile([128, 4 * CHUNK], bf16, tag="xn2T", name="xn2T")
                for t in range(TT_CH):
                    mv2, rs2 = layer_norm_rstd(x3[t], "2")
                    xc2 = xcp.tile([128, DIM], f32r, tag=f"xc2_{t}", name=f"xc2_{t}")
                    nc.vector.tensor_scalar(out=xc2, in0=x3[t], scalar1=mv2[:, 0:1],
                                            scalar2=rs2, op0=OP.subtract, op1=OP.mult)
                    TP2 = pstr.tile([128, 512], f32r, tag="tr", name="tp2")
                    for c in range(4):
                        nc.tensor.transpose(TP2[:, 128 * c:128 * (c + 1)],
                                            xc2[:, 128 * c:128 * (c + 1)], IDTR)
                    nc.scalar.activation(
                        out=r3(xn2T, 4)[:, :, 128 * t:128 * (t + 1)],
                        in_=r3(TP2.bitcast(f32), 4), func=AF.Copy)

                if k_stage < 7:
                    continue
                # ---- MLP ----
                h1 = [h1Tp.tile([128, CHUNK], bf16, tag=f"h1_{o}", name=f"h1_{o}") for o in range(16)]
                for o in range(16):
                    ps = psmm.tile([128, CHUNK], f32, tag="mm", name="mm")
                    for ci in range(4):
                        nc.tensor.matmul(ps, W1[ci][:, 128 * o:128 * (o + 1)],
                                         xn2T[:, CHUNK * ci:CHUNK * (ci + 1)],
                                         start=(ci == 0), stop=(ci == 3))
                    nc.scalar.activation(out=h1[o], in_=ps, func=AF.Gelu,
                                         bias=B1[:, o:o + 1])
                if k_stage < 8:
                    continue
                h2T = [h2Tp.tile([128, CHUNK], f32r, tag=f"h2T{c}", name=f"h2T{c}") for c in range(4)]
                for c in range(4):
                    ps = psmm.tile([128, CHUNK], f32, tag="mm", name="mm")
                    for hi in range(16):
                        nc.tensor.matmul(ps, W2b[hi][:, 128 * c:128 * (c + 1)],
                                         h1[hi], start=(hi == 0), stop=(hi == 15))
                    nc.scalar.activation(out=h2T[c], in_=ps, func=AF.Identity,
                                         bias=B2[:, c:c + 1])

                if k_stage < 9:
                    continue
                # ---- final add + store ----
                for t in range(TT_CH):
                    TY = pstr.tile([128, 512], f32r, tag="tr", name="ty")
                    for c in range(4):
                        nc.tensor.transpose(TY[:, 128 * c:128 * (c + 1)],
                                            h2T[c][:, 128 * t:128 * (t + 1)], IDTR)
                    yo = yop.tile([128, DIM], f32, tag=f"yo{t}", name=f"yo{t}")
                    nc.vector.tensor_tensor(out=yo, in0=TY.bitcast(f32), in1=x3[t],
                                            op=OP.add)
                    nc.gpsimd.dma_start(out=y_d[b, 256 * qt + 128 * t: 256 * qt + 128 * (t + 1), :],
                                      in_=yo)

    nc.compile()
    return nc


def _host_consts(rel_table):
    idx = _rel_index(WS).reshape(-1)
    bias = rel_table.reshape(-1, NH)[idx].reshape(N, NH, N)  # [n, h, m]
    qmask = _shift_mask(WS, SHIFT)                           # [64] True=masked
    keep = (~qmask).astype(np.float32)
    biasT = np.full((NH, 128, 128), NEG, np.float32)
    for h in range(NH):
        bT = bias[:, h, :].T * keep[None, :]                 # [m, n] masked cols->0
        biasT[h, :64, :64] = bT
        biasT[h, 64:, 64:] = bT
    # group per 4 heads side by side: [2, 128, 512]
    biasG = np.concatenate([
        biasT[4 * g:4 * (g + 1)].transpose(1, 0, 2).reshape(1, 128, 512)
        for g in range(2)], axis=0)
    qm = (np.tile(keep, CHUNK // N)[None, :].repeat(128, 0) * SCALE).astype(np.float32)
    return biasG, qm


def _win_order_sigmoid_gate(gate):
    g = 1.0 / (1.0 + np.exp(-gate.reshape(HRES, WRES).astype(np.float64)))
    g = g.astype(np.float32)
    sg = np.zeros((16, 64), np.float32)
    for w in range(16):
        wi, wj = w // 4, w % 4
        for i in range(8):
            for j in range(8):
                sg[w, 8 * i + j] = g[(8 * wi + i + 4) % 32, (8 * wj + j + 4) % 32]
    return sg.reshape(8, 128)


_PERM = None


def _win_pieces(w):
    wi, wj = w // 4, w % 4
    ih = [(0, 8, 8 * wi + 4)] if wi < 3 else [(0, 4, 28), (4, 4, 0)]
    jw = [(0, 8, 8 * wj + 4)] if wj < 3 else [(0, 4, 28), (4, 4, 0)]
    out = []
    for (i0, ni, h0) in ih:
        for (j0, nj, w0) in jw:
            out.append((i0, ni, h0, j0, nj, w0))
    return out


def _perm_idx():
    global _PERM
    if _PERM is None:
        p = np.zeros(1024, np.int64)
        for w in range(16):
            for (i0, ni, h0, j0, nj, w0) in _win_pieces(w):
                for a in range(ni):
                    for bb in range(nj):
                        p[64 * w + 8 * (i0 + a) + (j0 + bb)] = (h0 + a) * WRES + (w0 + bb)
        _PERM = p
    return _PERM


def kernel(**inputs):
    from concourse.bass_utils import run_bass_kernel_spmd

    bf = ml_dtypes.bfloat16
    x = np.asarray(inputs["x"], np.float32)           # (64,1,32,32,512)
    g1 = np.asarray(inputs["ln1_g"], np.float32)
    bl1 = np.asarray(inputs["ln1_b"], np.float32)
    g2 = np.asarray(inputs["ln2_g"], np.float32)
    bl2 = np.asarray(inputs["ln2_b"], np.float32)
    wq = np.asarray(inputs["wq"], np.float32)
    wk = np.asarray(inputs["wk"], np.float32)
    wv = np.asarray(inputs["wv"], np.float32)
    wp = np.asarray(inputs["wp"], np.float32)
    w1 = np.asarray(inputs["mlp_w1"], np.float32)
    w2 = np.asarray(inputs["mlp_w2"], np.float32)
    bq = np.asarray(inputs["bq"], np.float32)
    bk = np.asarray(inputs["bk"], np.float32)
    bv = np.asarray(inputs["bv"], np.float32)
    bp = np.asarray(inputs["bp"], np.float32)
    b1 = np.asarray(inputs["mlp_b1"], np.float32)
    b2 = np.asarray(inputs["mlp_b2"], np.float32)

    # LN affine folds
    wq_eff = wq * g1[None, :]
    wk_eff = wk * g1[None, :]
    wv_eff = wv * g1[None, :]
    bq_eff = bq + wq @ bl1
    bk_eff = bk + wk @ bl1
    bv_eff = bv + wv @ bl1
    w1_eff = w1 * g2[None, :]
    b1_eff = b1 + w1 @ bl2
    # attention-path channel constant: x2 = g1*xn + proj_raw + bconst
    bconst = bp + wp @ bv_eff + bl1
    bc_diag = np.zeros((4, 128, 128), np.float32)
    for c in range(4):
        np.fill_diagonal(bc_diag[c], bconst[128 * c:128 * (c + 1)])

    biasG, qm = _host_consts(np.asarray(inputs["rel_table"], np.float32))
    sgw = _win_order_sigmoid_gate(np.asarray(inputs["gate"], np.float32))
    common = {
        "wqT": np.ascontiguousarray(wq_eff.T).astype(bf),
        "wkT": np.ascontiguousarray(wk_eff.T).astype(bf),
        "wvT": np.ascontiguousarray(wv_eff.T).astype(bf),
        "wpT": np.ascontiguousarray(wp.T).astype(bf),
        "w1T": np.ascontiguousarray(w1_eff.T).astype(bf),
        "w2T": np.ascontiguousarray(w2.T).astype(bf),
        "bq": bq_eff,
        "bke": bk_eff * np.tile(np.r_[np.ones(64), np.zeros(64)], 4).astype(np.float32),
        "bko": bk_eff,
        "msk": np.stack([np.r_[np.ones(64), np.zeros(64)],
                         np.r_[np.zeros(64), np.ones(64)]], axis=1).astype(np.float32),
        "b1": b1_eff, "b2": b2,
        "g1": g1,
        "bc": bc_diag.astype(bf),
        "biasG": biasG.astype(bf), "qm": qm.astype(bf), "sgw": sgw,
        "idt": np.eye(128, dtype=np.float32),
    }
    if "prog" not in _prog_cache:
        _prog_cache["prog"] = _build_program()
    nc = _prog_cache["prog"]

    perm = _perm_idx()
    xw = x.reshape(B_TOTAL, TOK_IMG, DIM)[:, perm, :]   # window-ordered
    in_maps = []
    for c in range(NCORES):
        m = dict(common)
        m["x"] = np.ascontiguousarray(xw[c * B_LOC:(c + 1) * B_LOC])
        in_maps.append(m)
    res = run_bass_kernel_spmd(nc, in_maps, core_ids=list(range(NCORES)))
    yw = np.concatenate([res.results[c]["y"] for c in range(NCORES)], axis=0)
    out = np.empty((B_TOTAL, TOK_IMG, DIM), np.float32)
    out[:, perm, :] = yw
    return out.reshape(B_TOTAL, 1, HRES, WRES, DIM).astype(np.float32)


# revision 42
# speedup vs baseline: 1.4824x; 1.1436x over previous
"""CloudCastV2 shifted-window transformer block on 8 trn2 NeuronCores.

Data-parallel over batch: 64 images -> 8 per core. Each core runs the full
block (LN1 -> shifted-window MHA -> gated residual -> LN2 -> MLP -> residual)
on its 8 images. The (-4,-4) roll + 8x8 window partition is folded into the
input/output DMA access patterns, so on chip everything lives in
"window-ordered" token space (8192 tokens x 512 ch per core).

Key structure (v2):
  - LN affines folded into the QKV/MLP weights on the host; the attention-path
    per-channel constant (bp + Wp@bv_eff + ln1_b) is injected into the proj
    PSUM via a rank-1 ones matmul, so PSUM evictions are single fused ops.
  - All weights + on-chip activations bf16 except the residual stream (f32).
  - Attention batched per 4-head group: qk^T lands in one [128,512] PSUM bank,
    bias added in-place (DVE), one exp (Act) per group.
  - The 4 per-channel-block transposes of each 128-token tile share one
    [128,512] PSUM bank and leave via one strided eviction op.
  - rstd = exp(-0.5*ln(var+eps)) keeps LN + attention exp in one activation
    table set; only Gelu swaps tables (2 swaps/chunk).
  - Input/output DMAs issued from the SP engine (HWDGE), weights from gpsimd.
"""

import numpy as np
import ml_dtypes

WS, SHIFT, HEADS, DIM, HRES, WRES = 8, 4, 8, 512, 32, 32
N = WS * WS            # 64 tokens / window
NH = HEADS
D = DIM // NH          # 64
B_TOTAL, NCORES = 64, 8
B_LOC = B_TOTAL // NCORES          # 8 images / core
TOK_IMG = HRES * WRES              # 1024
CHUNK = 256                        # tokens per chunk (4 windows)
NCHUNK = B_LOC * TOK_IMG // CHUNK  # 32
TT_CH = CHUNK // 128               # 128-token tiles per chunk (2)
WP_CH = TT_CH                      # window-pairs per chunk (2)
SCALE = float(D) ** -0.5
NEG = -1.0e30

_prog_cache = {}


def _rel_index(ws):
    coords = np.arange(ws)
    grid = np.stack(np.meshgrid(coords, coords, indexing="ij"))
    flat = grid.reshape(2, -1)
    rel = flat[:, :, None] - flat[:, None, :]
    rel[0] += ws - 1
    rel[1] += ws - 1
    return rel[0] * (2 * ws - 1) + rel[1]


def _shift_mask(ws, shift):
    base = np.zeros((ws, ws), dtype=bool)
    base[ws - shift:, :] = True
    base[:, ws - shift:] = True
    return base.reshape(-1)


def _build_program():
    import concourse.bass as bass
    from concourse import bacc
    import concourse.mybir as mybir
    import concourse.tile as tile
    from concourse.masks import make_identity

    dt = mybir.dt
    f32, f32r, bf16 = dt.float32, dt.float32r, dt.bfloat16
    AF = mybir.ActivationFunctionType
    OP = mybir.AluOpType

    nc = bacc.Bacc("TRN2", target_bir_lowering=False, debug=True)
    x_d = nc.declare_dram_parameter("x", [B_LOC, TOK_IMG, DIM], f32, isOutput=False)
    y_d = nc.declare_dram_parameter("y", [B_LOC, TOK_IMG, DIM], f32, isOutput=True)
    wqT_d = nc.declare_dram_parameter("wqT", [DIM, DIM], bf16, isOutput=False)
    wkT_d = nc.declare_dram_parameter("wkT", [DIM, DIM], bf16, isOutput=False)
    wvT_d = nc.declare_dram_parameter("wvT", [DIM, DIM], bf16, isOutput=False)
    wpT_d = nc.declare_dram_parameter("wpT", [DIM, DIM], bf16, isOutput=False)
    w1T_d = nc.declare_dram_parameter("w1T", [DIM, 4 * DIM], bf16, isOutput=False)
    w2T_d = nc.declare_dram_parameter("w2T", [4 * DIM, DIM], bf16, isOutput=False)
    bq_d = nc.declare_dram_parameter("bq", [DIM], f32, isOutput=False)
    bke_d = nc.declare_dram_parameter("bke", [DIM], f32, isOutput=False)
    bko_d = nc.declare_dram_parameter("bko", [DIM], f32, isOutput=False)
    msk_d = nc.declare_dram_parameter("msk", [128, 2], f32, isOutput=False)
    b1_d = nc.declare_dram_parameter("b1", [4 * DIM], f32, isOutput=False)
    b2_d = nc.declare_dram_parameter("b2", [DIM], f32, isOutput=False)
    g1_d = nc.declare_dram_parameter("g1", [DIM], f32, isOutput=False)
    bc_d = nc.declare_dram_parameter("bc", [4, 128, 128], bf16, isOutput=False)  # diag(bconst)
    biasG_d = nc.declare_dram_parameter("biasG", [2, 128, 512], bf16, isOutput=False)
    qm_d = nc.declare_dram_parameter("qm", [128, CHUNK], bf16, isOutput=False)
    idt_d = nc.declare_dram_parameter("idt", [128, 128], f32, isOutput=False)
    sgw_d = nc.declare_dram_parameter("sgw", [8, 128], f32, isOutput=False)

    from contextlib import ExitStack

    with tile.TileContext(nc) as tc:
        with ExitStack() as es:
            P = lambda *a, **kw: es.enter_context(tc.tile_pool(*a, **kw))
            wts = P(name="wts", bufs=1)
            cst = P(name="cst", bufs=1)
            lnp = P(name="ln", bufs=4)
            xrp = P(name="xr", bufs=3)
            xcp = P(name="xc", bufs=2)
            xnTp = P(name="xnT", bufs=2)
            qkvp = P(name="qkv", bufs=2)
            ptp = P(name="pt", bufs=2)
            t3p = P(name="t3", bufs=2)
            rcp = P(name="rc", bufs=2)
            pnp = P(name="pn", bufs=2)
            aoTp = P(name="aoT", bufs=2)
            x2Tp = P(name="x2T", bufs=2)
            x3p = P(name="x3", bufs=2)
            xn2Tp = P(name="xn2T", bufs=2)
            h1Tp = P(name="h1T", bufs=2)
            h2Tp = P(name="h2T", bufs=2)
            yop = P(name="yo", bufs=2)
            # PSUM: 8 banks total. mm 3 (big matmuls + dn), qk 2,
            # tp1 1 (LN1 transposes), tr 2 (av/TX/TP2/TY).
            psmm = P(name="psmm", bufs=3, space="PSUM")
            psqk = P(name="psqk", bufs=2, space="PSUM")
            pstp1 = P(name="pstp1", bufs=1, space="PSUM")
            pstr = P(name="pstr", bufs=2, space="PSUM")

            # ---- resident weights & constants ----
            WQ = [wts.tile([128, DIM], bf16, name=f"wq{i}") for i in range(4)]
            WK = [wts.tile([128, DIM], bf16, name=f"wk{i}") for i in range(4)]
            WV = [wts.tile([128, DIM], bf16, name=f"wv{i}") for i in range(4)]
            WP = [wts.tile([128, DIM], bf16, name=f"wp{i}") for i in range(4)]
            W1 = [wts.tile([128, 4 * DIM], bf16, name=f"w1{i}") for i in range(4)]
            for i in range(4):
                nc.gpsimd.dma_start(out=WQ[i], in_=wqT_d[128 * i:128 * (i + 1), :])
                nc.gpsimd.dma_start(out=WK[i], in_=wkT_d[128 * i:128 * (i + 1), :])
                nc.gpsimd.dma_start(out=WV[i], in_=wvT_d[128 * i:128 * (i + 1), :])
                nc.gpsimd.dma_start(out=WP[i], in_=wpT_d[128 * i:128 * (i + 1), :])
                nc.gpsimd.dma_start(out=W1[i], in_=w1T_d[128 * i:128 * (i + 1), :])
            W2b = [wts.tile([128, DIM], bf16, name=f"w2b{i}") for i in range(16)]
            for i in range(16):
                nc.gpsimd.dma_start(out=W2b[i], in_=w2T_d[128 * i:128 * (i + 1), :])

            BIASG = [cst.tile([128, 512], bf16, name=f"biasg{g}") for g in range(2)]
            for g in range(2):
                nc.gpsimd.dma_start(out=BIASG[g], in_=biasG_d[g])
            QM = cst.tile([128, CHUNK], bf16, name="qm")
            nc.gpsimd.dma_start(out=QM, in_=qm_d[:, :])
            SG = cst.tile([128, 8], f32, name="sg")
            nc.gpsimd.dma_start(out=SG, in_=sgw_d[:, :].rearrange("t p -> p t"))
            BC = [cst.tile([128, 128], bf16, name=f"bc{c}") for c in range(4)]
            for c in range(4):
                nc.gpsimd.dma_start(out=BC[c], in_=bc_d[c])
            IDTB = cst.tile([128, 128], bf16, name="idtb")
            make_identity(nc, IDTB)
            IDTR = cst.tile([128, 128], f32r, name="idtr")
            nc.gpsimd.dma_start(out=IDTR, in_=idt_d[:, :].bitcast(f32r))
            ONES = cst.tile([128, 128], bf16, name="ones")
            nc.vector.memset(ONES, 1.0)
            ONES2 = cst.tile([128, CHUNK], bf16, name="ones2")
            nc.vector.memset(ONES2, 1.0)
            EPS = cst.tile([128, 1], f32, name="eps")
            nc.vector.memset(EPS, 1e-5)

            def vec_sb(dram, n, name):
                t = cst.tile([128, n], f32, name=name)
                nc.gpsimd.dma_start(out=t, in_=dram[:].rearrange("(t p) -> p t", p=128))
                return t

            BQ = vec_sb(bq_d, 4, "bq")
            BKE = vec_sb(bke_d, 4, "bke")
            BKO = vec_sb(bko_d, 4, "bko")
            MSK = cst.tile([128, 2], f32, name="msk")
            nc.gpsimd.dma_start(out=MSK, in_=msk_d[:, :])
            B1 = vec_sb(b1_d, 16, "b1")
            B2 = vec_sb(b2_d, 4, "b2")
            G1 = vec_sb(g1_d, 4, "g1")

            # One-time DVE "touch" of every DMA-loaded tile: converts all
            # weight/const readiness into vector-engine program order so no
            # downstream instruction needs more than 2 sync waits.
            scr = cst.tile([128, 2048], f32, name="scr")
            touch_list = (WQ + WK + WV + WP + W1 + W2b + BIASG + BC
                          + [QM, SG, BQ, BKE, BKO, MSK, B1, B2, G1])
            for tt_ in touch_list:
                n_ = tt_.shape[-1] if len(tt_.shape) == 2 else 1
                if tt_.dtype == bf16:
                    nc.vector.tensor_copy(out=scr.bitcast(bf16)[:tt_.shape[0], :n_], in_=tt_)
                else:
                    nc.vector.tensor_copy(out=scr[:tt_.shape[0], :n_], in_=tt_)

            def layer_norm_rstd(xin, tag):
                """per-token mean + rstd of xin [128, DIM] via Ln/Exp."""
                st = lnp.tile([128, 6], f32, tag=f"st{tag}", name=f"st{tag}")
                nc.vector.bn_stats(out=st, in_=xin)
                mv = lnp.tile([128, 2], f32, tag=f"mv{tag}", name=f"mv{tag}")
                nc.vector.bn_aggr(out=mv, in_=st)
                sd = lnp.tile([128, 1], f32, tag=f"sd{tag}", name=f"sd{tag}")
                nc.scalar.activation(out=sd, in_=mv[:, 1:2], func=AF.Sqrt, bias=EPS)
                rs = lnp.tile([128, 1], f32, tag=f"rs{tag}", name=f"rs{tag}")
                nc.vector.reciprocal(out=rs, in_=sd)
                return mv, rs

            def r3(t, c=4):
                return t.rearrange("p (c q) -> p c q", c=c)

            import os
            n_chunks = int(os.environ.get("K_NCHUNK", str(NCHUNK)))
            k_stage = int(os.environ.get("K_STAGE", "9"))
            for ch in range(n_chunks):
                b, qt = ch // 4, ch % 4

                # ---- load (window-ordered) + LN1 + transpose -> xnT ----
                xr = [xrp.tile([128, DIM], f32, tag=f"xr{t}", name=f"xr{t}") for t in range(TT_CH)]
                for t in range(TT_CH):
                    nc.gpsimd.dma_start(out=xr[t], in_=x_d[b, 256 * qt + 128 * t: 256 * qt + 128 * (t + 1), :])
                # xnT: [128, 1024] bf16, c-major blocks of 256 (= 2 t-tiles of 128)
                xnT = xnTp.tile([128, 4 * CHUNK], bf16, tag="xnT", name="xnT")
                for t in range(TT_CH):
                    mv, rs = layer_norm_rstd(xr[t], "1")
                    xc = xcp.tile([128, DIM], f32r, tag=f"xc{t}", name=f"xc{t}")
                    nc.vector.tensor_scalar(out=xc, in0=xr[t], scalar1=mv[:, 0:1],
                                            scalar2=rs, op0=OP.subtract, op1=OP.mult)
                    TP = pstp1.tile([128, 512], f32r, tag="tp1", name="tp1")
                    for c in range(4):
                        nc.tensor.transpose(TP[:, 128 * c:128 * (c + 1)],
                                            xc[:, 128 * c:128 * (c + 1)], IDTR)
                    # one strided eviction: TP c-blocks -> xnT[:, 256c+128t : +128]
                    nc.scalar.activation(
                        out=r3(xnT, 4)[:, :, 128 * t:128 * (t + 1)],
                        in_=r3(TP.bitcast(f32), 4), func=AF.Copy)

                def xnTc(c):
                    return xnT[:, CHUNK * c:CHUNK * (c + 1)]

                if k_stage < 2:
                    continue

                # ---- QKV ----
                qT = [qkvp.tile([128, CHUNK], bf16, tag=f"qT{c}", name=f"qT{c}") for c in range(4)]
                kTE = [qkvp.tile([128, CHUNK], bf16, tag=f"kTE{c}", name=f"kTE{c}") for c in range(4)]
                kTO = [qkvp.tile([128, CHUNK], bf16, tag=f"kTO{c}", name=f"kTO{c}") for c in range(4)]
                vN = [qkvp.tile([128, DIM], bf16, tag=f"vN{t}", name=f"vN{t}") for t in range(TT_CH)]
                for c in range(4):
                    ps = psmm.tile([128, CHUNK], f32, tag="mm", name="mm")
                    for ci in range(4):
                        nc.tensor.matmul(ps, WQ[ci][:, 128 * c:128 * (c + 1)],
                                         xnTc(ci), start=(ci == 0), stop=(ci == 3))
                    # qT = (ps + bq) * (SCALE*keep)
                    nc.vector.scalar_tensor_tensor(
                        out=qT[c], in0=ps, scalar=BQ[:, c:c + 1], in1=QM,
                        op0=OP.add, op1=OP.mult)
                    ps2 = psmm.tile([128, CHUNK], f32, tag="mm", name="mm")
                    for ci in range(4):
                        nc.tensor.matmul(ps2, WK[ci][:, 128 * c:128 * (c + 1)],
                                         xnTc(ci), start=(ci == 0), stop=(ci == 3))
                    # parity-masked kT: even-head rows / odd-head rows only,
                    # so qk matmuls can contract over the full 128 partitions
                    nc.scalar.activation(out=kTE[c], in_=ps2, func=AF.Identity,
                                         scale=MSK[:, 0:1], bias=BKE[:, c:c + 1])
                    nc.vector.tensor_scalar(out=kTO[c], in0=ps2,
                                            scalar1=BKO[:, c:c + 1],
                                            scalar2=MSK[:, 1:2],
                                            op0=OP.add, op1=OP.mult)
                for t in range(TT_CH):
                    ps = psmm.tile([128, DIM], f32, tag="mm", name="mm")
                    for ci in range(4):
                        nc.tensor.matmul(ps, xnT[:, CHUNK * ci + 128 * t: CHUNK * ci + 128 * (t + 1)],
                                         WV[ci], start=(ci == 0), stop=(ci == 3))
                    nc.scalar.activation(out=vN[t], in_=ps, func=AF.Copy)

                if k_stage < 3:
                    continue
                k_attn = int(os.environ.get("K_ATTN", "4"))
                # ---- attention ----
                # aoT: [128, 1024] bf16, c-major blocks of 256 (= 2 wp-tiles of 128)
                aoT = aoTp.tile([128, 4 * CHUNK], bf16, tag="aoT", name="aoT")
                for wp in range(WP_CH):
                    PT = ptp.tile([128, NH * 128], bf16, tag="pt", name="pt")
                    sl = slice(128 * wp, 128 * (wp + 1))
                    for g in range(2):
                        qk = psqk.tile([128, 512], f32, tag="qk", name="qk")
                        for hh in range(4):
                            h = 4 * g + hh
                            cth = h // 2
                            kTz = kTE[cth] if h % 2 == 0 else kTO[cth]
                            nc.tensor.matmul(qk[:, 128 * hh:128 * (hh + 1)],
                                             kTz[:, sl], qT[cth][:, sl],
                                             start=True, stop=True)
                        # bias add -> SBUF, then one exp for the 4-head group
                        t3 = t3p.tile([128, 512], f32, tag="t3", name="t3")
                        nc.vector.tensor_tensor(out=t3, in0=qk, in1=BIASG[g],
                                                op=OP.add)
                        nc.scalar.activation(out=PT[:, 512 * g:512 * (g + 1)],
                                             in_=t3, func=AF.Exp)
                    if k_attn < 2:
                        continue
                    pn = pnp.tile([128, NH * 128], bf16, tag="pn", name="pn")
                    for g in range(2):
                        dn = psmm.tile([128, 512], f32, tag="mm", name="mm")
                        nc.tensor.matmul(dn, ONES, PT[:, 512 * g:512 * (g + 1)],
                                         start=True, stop=True)
                        r = rcp.tile([128, 512], bf16, tag=f"rc{g}", name=f"rc{g}")
                        with nc.allow_low_precision(reason="attn weights bf16"):
                            nc.vector.reciprocal(out=r, in_=dn)
                        if k_attn < 3:
                            continue
                        nc.gpsimd.tensor_mul(out=pn[:, 512 * g:512 * (g + 1)],
                                             in0=PT[:, 512 * g:512 * (g + 1)], in1=r)
                    if k_attn < 4:
                        continue
                    av = pstr.tile([128, 512], f32, tag="tr", name="av")
                    for h in range(NH):
                        cth, ro = h // 2, 64 * (h % 2)
                        nc.tensor.matmul(av[ro:ro + 64, 128 * cth:128 * (cth + 1)],
                                         vN[wp][:, 64 * h:64 * (h + 1)],
                                         pn[:, 128 * h:128 * (h + 1)],
                                         start=True, stop=True,
                                         tile_position=(0, ro))
                    nc.scalar.activation(
                        out=r3(aoT, 4)[:, :, 128 * wp:128 * (wp + 1)],
                        in_=r3(av, 4), func=AF.Copy)

                if k_stage < 4:
                    continue
                # ---- proj + rank-1 bias + residual (in T) ----
                x2T = [x2Tp.tile([128, CHUNK], f32r, tag=f"x2T{c}", name=f"x2T{c}") for c in range(4)]
                for c in range(4):
                    ps = psmm.tile([128, CHUNK], f32, tag="mm", name="mm")
                    for ci in range(4):
                        nc.tensor.matmul(ps, WP[ci][:, 128 * c:128 * (c + 1)],
                                         aoT[:, CHUNK * ci:CHUNK * (ci + 1)],
                                         start=(ci == 0), stop=False)
                    nc.tensor.matmul(ps, BC[c], ONES2,
                                     start=False, stop=True)
                    # x2T = g1 (.) xnT + (proj + bconst)
                    nc.vector.scalar_tensor_tensor(
                        out=x2T[c], in0=xnTc(c), scalar=G1[:, c:c + 1], in1=ps,
                        op0=OP.mult, op1=OP.add)

                if k_stage < 5:
                    continue
                # ---- back to natural: x3 = x2 + sig(gate)*x ----
                x3 = [x3p.tile([128, DIM], f32, tag=f"x3{t}", name=f"x3{t}") for t in range(TT_CH)]
                for t in range(TT_CH):
                    TX = pstr.tile([128, 512], f32r, tag="tr", name="tx")
                    for c in range(4):
                        nc.tensor.transpose(TX[:, 128 * c:128 * (c + 1)],
                                            x2T[c][:, 128 * t:128 * (t + 1)],
                                            IDTR)
                    col = 2 * qt + t
                    nc.vector.scalar_tensor_tensor(
                        out=x3[t], in0=xr[t], scalar=SG[:, col:col + 1],
                        in1=TX.bitcast(f32), op0=OP.mult, op1=OP.add)

                if k_stage < 6:
                    continue
                # ---- LN2 + transpose (g2/b2 folded into W1/b1) ----
                xn2T = xn2Tp.tile([128, 4 * CHUNK], bf16, tag="xn2T", name="xn2T")
                for t in range(TT_CH):
                    mv2, rs2 = layer_norm_rstd(x3[t], "2")
                    xc2 = xcp.tile([128, DIM], f32r, tag=f"xc2_{t}", name=f"xc2_{t}")
                    nc.vector.tensor_scalar(out=xc2, in0=x3[t], scalar1=mv2[:, 0:1],
                                            scalar2=rs2, op0=OP.subtract, op1=OP.mult)
                    TP2 = pstr.tile([128, 512], f32r, tag="tr", name="tp2")
                    for c in range(4):
                        nc.tensor.transpose(TP2[:, 128 * c:128 * (c + 1)],
                                            xc2[:, 128 * c:128 * (c + 1)], IDTR)
                    nc.scalar.activation(
                        out=r3(xn2T, 4)[:, :, 128 * t:128 * (t + 1)],
                        in_=r3(TP2.bitcast(f32), 4), func=AF.Copy)

                if k_stage < 7:
                    continue
                # ---- MLP ----
                h1 = [h1Tp.tile([128, CHUNK], bf16, tag=f"h1_{o}", name=f"h1_{o}") for o in range(16)]
                for o in range(16):
                    ps = psmm.tile([128, CHUNK], f32, tag="mm", name="mm")
                    for ci in range(4):
                        nc.tensor.matmul(ps, W1[ci][:, 128 * o:128 * (o + 1)],
                                         xn2T[:, CHUNK * ci:CHUNK * (ci + 1)],
                                         start=(ci == 0), stop=(ci == 3))
                    nc.scalar.activation(out=h1[o], in_=ps, func=AF.Gelu,
                                         bias=B1[:, o:o + 1])
                if k_stage < 8:
                    continue
                h2T = [h2Tp.tile([128, CHUNK], f32r, tag=f"h2T{c}", name=f"h2T{c}") for c in range(4)]
                for c in range(4):
                    ps = psmm.tile([128, CHUNK], f32, tag="mm", name="mm")
                    for hi in range(16):
                        nc.tensor.matmul(ps, W2b[hi][:, 128 * c:128 * (c + 1)],
                                         h1[hi], start=(hi == 0), stop=(hi == 15))
                    nc.scalar.activation(out=h2T[c], in_=ps, func=AF.Identity,
                                         bias=B2[:, c:c + 1])

                if k_stage < 9:
                    continue
                # ---- final add + store ----
                for t in range(TT_CH):
                    TY = pstr.tile([128, 512], f32r, tag="tr", name="ty")
                    for c in range(4):
                        nc.tensor.transpose(TY[:, 128 * c:128 * (c + 1)],
                                            h2T[c][:, 128 * t:128 * (t + 1)], IDTR)
                    yo = yop.tile([128, DIM], f32, tag=f"yo{t}", name=f"yo{t}")
                    nc.vector.tensor_tensor(out=yo, in0=TY.bitcast(f32), in1=x3[t],
                                            op=OP.add)
                    nc.gpsimd.dma_start(out=y_d[b, 256 * qt + 128 * t: 256 * qt + 128 * (t + 1), :],
                                      in_=yo)

    nc.compile()
    return nc


def _host_consts(rel_table):
    idx = _rel_index(WS).reshape(-1)
    bias = rel_table.reshape(-1, NH)[idx].reshape(N, NH, N)  # [n, h, m]
    qmask = _shift_mask(WS, SHIFT)                           # [64] True=masked
    keep = (~qmask).astype(np.float32)
    biasT = np.full((NH, 128, 128), NEG, np.float32)
    for h in range(NH):
        bT = bias[:, h, :].T * keep[None, :]                 # [m, n] masked cols->0
        biasT[h, :64, :64] = bT
        biasT[h, 64:, 64:] = bT
    # group per 4 heads side by side: [2, 128, 512]
    biasG = np.concatenate([
        biasT[4 * g:4 * (g + 1)].transpose(1, 0, 2).reshape(1, 128, 512)
        for g in range(2)], axis=0)
    qm = (np.tile(keep, CHUNK // N)[None, :].repeat(128, 0) * SCALE).astype(np.float32)
    return biasG, qm


def _win_order_sigmoid_gate(gate):
    g = 1.0 / (1.0 + np.exp(-gate.reshape(HRES, WRES).astype(np.float64)))
    g = g.astype(np.float32)
    sg = np.zeros((16, 64), np.float32)
    for w in range(16):
        wi, wj = w // 4, w % 4
        for i in range(8):
            for j in range(8):
                sg[w, 8 * i + j] = g[(8 * wi + i + 4) % 32, (8 * wj + j + 4) % 32]
    return sg.reshape(8, 128)


_PERM = None


def _win_pieces(w):
    wi, wj = w // 4, w % 4
    ih = [(0, 8, 8 * wi + 4)] if wi < 3 else [(0, 4, 28), (4, 4, 0)]
    jw = [(0, 8, 8 * wj + 4)] if wj < 3 else [(0, 4, 28), (4, 4, 0)]
    out = []
    for (i0, ni, h0) in ih:
        for (j0, nj, w0) in jw:
            out.append((i0, ni, h0, j0, nj, w0))
    return out


def _perm_idx():
    global _PERM
    if _PERM is None:
        p = np.zeros(1024, np.int64)
        for w in range(16):
            for (i0, ni, h0, j0, nj, w0) in _win_pieces(w):
                for a in range(ni):
                    for bb in range(nj):
                        p[64 * w + 8 * (i0 + a) + (j0 + bb)] = (h0 + a) * WRES + (w0 + bb)
        _PERM = p
    return _PERM


def kernel(**inputs):
    from concourse.bass_utils import run_bass_kernel_spmd

    bf = ml_dtypes.bfloat16
    x = np.asarray(inputs["x"], np.float32)           # (64,1,32,32,512)
    g1 = np.asarray(inputs["ln1_g"], np.float32)
    bl1 = np.asarray(inputs["ln1_b"], np.float32)
    g2 = np.asarray(inputs["ln2_g"], np.float32)
    bl2 = np.asarray(inputs["ln2_b"], np.float32)
    wq = np.asarray(inputs["wq"], np.float32)
    wk = np.asarray(inputs["wk"], np.float32)
    wv = np.asarray(inputs["wv"], np.float32)
    wp = np.asarray(inputs["wp"], np.float32)
    w1 = np.asarray(inputs["mlp_w1"], np.float32)
    w2 = np.asarray(inputs["mlp_w2"], np.float32)
    bq = np.asarray(inputs["bq"], np.float32)
    bk = np.asarray(inputs["bk"], np.float32)
    bv = np.asarray(inputs["bv"], np.float32)
    bp = np.asarray(inputs["bp"], np.float32)
    b1 = np.asarray(inputs["mlp_b1"], np.float32)
    b2 = np.asarray(inputs["mlp_b2"], np.float32)

    # LN affine folds
    wq_eff = wq * g1[None, :]
    wk_eff = wk * g1[None, :]
    wv_eff = wv * g1[None, :]
    bq_eff = bq + wq @ bl1
    bk_eff = bk + wk @ bl1
    bv_eff = bv + wv @ bl1
    w1_eff = w1 * g2[None, :]
    b1_eff = b1 + w1 @ bl2
    # attention-path channel constant: x2 = g1*xn + proj_raw + bconst
    bconst = bp + wp @ bv_eff + bl1
    bc_diag = np.zeros((4, 128, 128), np.float32)
    for c in range(4):
        np.fill_diagonal(bc_diag[c], bconst[128 * c:128 * (c + 1)])

    biasG, qm = _host_consts(np.asarray(inputs["rel_table"], np.float32))
    sgw = _win_order_sigmoid_gate(np.asarray(inputs["gate"], np.float32))
    common = {
        "wqT": np.ascontiguousarray(wq_eff.T).astype(bf),
        "wkT": np.ascontiguousarray(wk_eff.T).astype(bf),
        "wvT": np.ascontiguousarray(wv_eff.T).astype(bf),
        "wpT": np.ascontiguousarray(wp.T).astype(bf),
        "w1T": np.ascontiguousarray(w1_eff.T).astype(bf),
        "w2T": np.ascontiguousarray(w2.T).astype(bf),
        "bq": bq_eff,
        "bke": bk_eff * np.tile(np.r_[np.ones(64), np.zeros(64)], 4).astype(np.float32),
        "bko": bk_eff,
        "msk": np.stack([np.r_[np.ones(64), np.zeros(64)],
                         np.r_[np.zeros(64), np.ones(64)]], axis=1).astype(np.float32),
        "b1": b1_eff, "b2": b2,
        "g1": g1,
        "bc": bc_diag.astype(bf),
        "biasG": biasG.astype(bf), "qm": qm.astype(bf), "sgw": sgw,
        "idt": np.eye(128, dtype=np.float32),
    }
    if "prog" not in _prog_cache:
        _prog_cache["prog"] = _build_program()
    nc = _prog_cache["prog"]

    perm = _perm_idx()
    xw = x.reshape(B_TOTAL, TOK_IMG, DIM)[:, perm, :]   # window-ordered
    in_maps = []
    for c in range(NCORES):
        m = dict(common)
        m["x"] = np.ascontiguousarray(xw[c * B_LOC:(c + 1) * B_LOC])
        in_maps.append(m)
    res = run_bass_kernel_spmd(nc, in_maps, core_ids=list(range(NCORES)))
    yw = np.concatenate([res.results[c]["y"] for c in range(NCORES)], axis=0)
    out = np.empty((B_TOTAL, TOK_IMG, DIM), np.float32)
    out[:, perm, :] = yw
    return out.reshape(B_TOTAL, 1, HRES, WRES, DIM).astype(np.float32)
